# revision 16
# baseline (speedup 1.0000x reference)
"""GPT-2-like transformer forward on 8 Trainium2 NeuronCores.

Sharding: batch (4) -> core pairs; within a pair, sequence-parallel split of
the 8 query blocks of 128 (zigzag {0,3,4,7}/{1,2,5,6} for causal balance).
Per layer: one K/V AllGather inside each pair. lm_head vocab-sharded 8 ways
(tiny AllReduce of the 4 final-position vectors first). Matmuls in bf16 with
fp32 accumulation; residual stream fp32. LN gammas are folded into the
following weight matrices on the host (exact rewrite).
"""

import math
from contextlib import ExitStack

import numpy as np
import ml_dtypes

import concourse.bass as bass
import concourse.mybir as mybir
import concourse.tile as tile
from concourse import bacc
from concourse.masks import make_identity

BF16 = mybir.dt.bfloat16
F32 = mybir.dt.float32
I32 = mybir.dt.int32
AF = mybir.ActivationFunctionType
ALU = mybir.AluOpType

P = 128
NEG = -1.0e9

G_EVEN = [0, 3, 4, 7]
G_ODD = [1, 2, 5, 6]
# global k-tile g -> (rank in pair, slot in that rank's shard)
G2SLOT = {}
for _s, _g in enumerate(G_EVEN):
    G2SLOT[_g] = (0, _s)
for _s, _g in enumerate(G_ODD):
    G2SLOT[_g] = (1, _s)


def full_cfg():
    return dict(B=4, T=1024, D=1024, H=16, HD=64, L=12, DFF=4096, V=50257,
                NCORES=8, EPS=1e-5)


def derived(cfg):
    d = dict(cfg)
    d["NC"] = cfg["D"] // P              # dim chunks
    d["NBL"] = 4                          # local q blocks per core
    d["NKT"] = 8                          # global k tiles
    d["TL"] = d["NBL"] * P                # local tokens (512)
    d["NM"] = cfg["DFF"] // P             # mlp hidden chunks
    d["NMG"] = cfg["DFF"] // 512          # mlp hidden groups of 512
    d["VS"] = math.ceil(cfg["V"] / cfg["NCORES"])   # vocab shard
    d["NPAIR"] = cfg["NCORES"] // 2
    d["KSZ"] = d["NC"] * P * d["TL"]      # k elements in kv slab
    d["VSZ"] = d["NBL"] * P * cfg["D"]    # v elements in kv slab
    d["KVE"] = d["KSZ"] + d["VSZ"]
    return d


# --------------------------------------------------------------------------
# device program
# --------------------------------------------------------------------------

def build_program(cfg):
    c = derived(cfg)
    B, D, H, HD, L, DFF, NCORES = (c["B"], c["D"], c["H"], c["HD"], c["L"],
                                   c["DFF"], c["NCORES"])
    NC, NBL, NKT, TL, NM, NMG, VS = (c["NC"], c["NBL"], c["NKT"], c["TL"],
                                     c["NM"], c["NMG"], c["VS"])
    KSZ, KVE = c["KSZ"], c["KVE"]
    EPS = c["EPS"]
    NVT = math.ceil(VS / 512)             # vocab tiles per core
    DHS = [(i * 512, min(512, D - i * 512)) for i in range((D + 511) // 512)]
    HGRP = min(8, NM)                     # wpr hidden chunks per dma group
    NHG = (NM + HGRP - 1) // HGRP

    nc = bacc.Bacc("TRN2", target_bir_lowering=False, debug=False,
                   num_devices=NCORES)

    # ---- I/O ----
    idx_in = nc.declare_dram_parameter("x_idx", [NBL, P], I32, isOutput=False)
    wpe_in = nc.declare_dram_parameter("wpe_loc", [NBL, P, D], F32, isOutput=False)
    wte_emb = nc.declare_dram_parameter("wte_emb", [cfg["V"], D], BF16, isOutput=False)
    wq_all = nc.declare_dram_parameter("wq_all", [L, D, D], BF16, isOutput=False)
    wk_all = nc.declare_dram_parameter("wk_all", [L, D, D], BF16, isOutput=False)
    wv_all = nc.declare_dram_parameter("wv_all", [L, D, D], BF16, isOutput=False)
    wo_all = nc.declare_dram_parameter("wo_all", [L, D, D], BF16, isOutput=False)
    wfc_all = nc.declare_dram_parameter("wfc_all", [L, D, DFF], BF16, isOutput=False)
    wpr_all = nc.declare_dram_parameter("wpr_all", [L, DFF, D], BF16, isOutput=False)
    maskT_in = nc.declare_dram_parameter("maskT", [NKT, P, P], F32, isOutput=False)
    onehot_in = nc.declare_dram_parameter("onehot", [1, B], BF16, isOutput=False)
    wteT_in = nc.declare_dram_parameter("wteT_loc", [D, VS], BF16, isOutput=False)
    logits_out = nc.declare_dram_parameter("logits_loc", [B, VS], F32, isOutput=True)

    # ---- internal DRAM for collectives ----
    kv_in = nc.dram_tensor("kv_in", [KVE], BF16)
    kv_out = nc.dram_tensor("kv_out", [2, KVE], BF16)
    lm_in = nc.dram_tensor("lm_in", [B, D], F32)
    lm_out = nc.dram_tensor("lm_out", [B, D], F32,
                            addr_space="Shared" if NCORES > 4 else "Local")

    pair_groups = [[2 * p, 2 * p + 1] for p in range(c["NPAIR"])]
    all_group = [list(range(NCORES))]

    with tile.TileContext(nc) as tc, ExitStack() as ctx:
        persist = ctx.enter_context(tc.tile_pool(name="persist", bufs=1))
        wpool = ctx.enter_context(tc.tile_pool(name="wpool", bufs=2))
        sb3 = ctx.enter_context(tc.tile_pool(name="sb3", bufs=2))
        probs_pool = ctx.enter_context(tc.tile_pool(name="probs_pool", bufs=4))
        stat = ctx.enter_context(tc.tile_pool(name="stat", bufs=4))
        lm1 = ctx.enter_context(tc.tile_pool(name="lm1", bufs=1))

        # persistent state
        x = persist.tile([P, NBL, D], F32, tag="x")
        hT = persist.tile([P, NC, TL], BF16, tag="hT")
        qT = persist.tile([P, NC, TL], BF16, tag="qT")
        kT_full = persist.tile([P, NC, NKT * P], BF16, tag="kT_full")
        v_full = persist.tile([P, NKT, H, HD + 1], BF16, tag="v_full")
        ctxT = persist.tile([P, NC, TL], BF16, tag="ctxT")
        RM = min(8, NM)
        NROUND = NM // RM
        upool = ctx.enter_context(tc.tile_pool(name="upool", bufs=2))

        maskT = persist.tile([P, NKT, P], F32, tag="maskT")
        ident = persist.tile([P, P], BF16, tag="ident")
        ones_col = persist.tile([P, HD], F32, tag="ones_col")
        eps_t = persist.tile([P, 1], F32, tag="eps")

        make_identity(nc, ident)
        nc.vector.memset(ones_col, 1.0)
        nc.vector.memset(eps_t, EPS)
        # ones column of the augmented V (sums row of PV output)
        nc.vector.memset(v_full[:, :, :, HD:HD + 1], 1.0)
        nc.sync.dma_start(out=maskT, in_=maskT_in.rearrange("k p q -> p k q"))

        # ---- embeddings ----
        for t in range(NBL):
            idx_t = stat.tile([P, 1], I32, tag="idx")
            nc.sync.dma_start(out=idx_t, in_=idx_in[t, :].rearrange("(p o) -> p o", o=1))
            emb = sb3.tile([P, D], BF16, tag="h_bf")
            nc.gpsimd.indirect_dma_start(
                out=emb[:, :], out_offset=None, in_=wte_emb[:, :],
                in_offset=bass.IndirectOffsetOnAxis(ap=idx_t[:, 0:1], axis=0))
            wpe_t = sb3.tile([P, D], F32, tag="h_bf")
            nc.sync.dma_start(out=wpe_t, in_=wpe_in[t])
            nc.vector.tensor_tensor(out=x[:, t, :], in0=wpe_t[:, :],
                                    in1=emb[:, :], op=ALU.add)

        def layernorm(src_ap, dst_ap, plo=0, n_rows=P):
            # src [n_rows, D] fp32 -> dst [n_rows, D] bf16 (gamma folded into W)
            phi = plo + n_rows
            ngrp = max(1, D // 512)
            st = stat.tile([P, ngrp, 6], F32, tag="bnst")
            grp = src_ap.rearrange("p (g d) -> p g d", g=ngrp)
            for g in range(ngrp):
                nc.vector.bn_stats(out=st[plo:phi, g, :], in_=grp[:, g, :])
            mv = stat.tile([P, 2], F32, tag="bnmv")
            nc.vector.bn_aggr(out=mv[plo:phi], in_=st[plo:phi])
            rs = stat.tile([P, 1], F32, tag="bnrs")
            nc.scalar.activation(out=rs[plo:phi], in_=mv[plo:phi, 1:2],
                                 func=AF.Sqrt, bias=eps_t[plo:phi])
            nc.vector.reciprocal(out=rs[plo:phi], in_=rs[plo:phi])
            nc.vector.tensor_scalar(out=dst_ap, in0=src_ap,
                                    scalar1=mv[plo:phi, 0:1], scalar2=rs[plo:phi],
                                    op0=ALU.subtract, op1=ALU.mult)

        def transpose_to(h_bf, dstT, psp):
            # h_bf [P, NBL, D] bf16 token-major -> dstT [P, NC, TL] feature-major
            for t in range(NBL):
                for cc in range(NC):
                    tp = psp.tile([P, P], BF16, tag="tp")
                    nc.tensor.transpose(out=tp[:, :], in_=h_bf[:, t, cc * P:(cc + 1) * P],
                                        identity=ident[:, :])
                    nc.scalar.copy(out=dstT[:, cc, t * P:(t + 1) * P], in_=tp[:, :])

        def load_w(dram_l):
            w = wpool.tile([P, NC, D], BF16, tag="w4")
            nc.sync.dma_start(out=w, in_=dram_l.rearrange("(c p) o -> p c o", p=P))
            return w

        # ================= layers =================
        for l in range(L):
            h_bf = sb3.tile([P, NBL, D], BF16, tag="h_bf")
            with tc.tile_pool(name=f"ps_tp{l}", bufs=2, space="PSUM") as psp:
                for t in range(NBL):
                    layernorm(x[:, t, :], h_bf[:, t, :])
                transpose_to(h_bf, hT, psp)

            # ---- K, V, Q projections ----
            with tc.tile_pool(name=f"ps_kvq{l}", bufs=3, space="PSUM") as psp:
                vk_view = kv_in[0:KSZ].rearrange("(c p t) -> p c t", c=NC, p=P)
                vv_view = kv_in[KSZ:KVE].rearrange("(t p d) -> p t d", t=NBL, p=P)
                wk = load_w(wk_all[l])
                for co in range(NC):
                    ps = psp.tile([P, TL], F32, tag="mm")
                    for cc in range(NC):
                        nc.tensor.matmul(ps[:, :], wk[:, cc, co * P:(co + 1) * P],
                                         hT[:, cc, :], start=(cc == 0),
                                         stop=(cc == NC - 1))
                    kst = sb3.tile([P, TL], BF16, tag="kvst")
                    nc.scalar.copy(out=kst[:, :], in_=ps[:, :])
                    nc.sync.dma_start(out=vk_view[:, co, :], in_=kst)
                wv = load_w(wv_all[l])
                for t in range(NBL):
                    for off, dw in DHS:
                        ps = psp.tile([P, 512], F32, tag="mm2")
                        for cc in range(NC):
                            nc.tensor.matmul(ps[:, 0:dw], hT[:, cc, t * P:(t + 1) * P],
                                             wv[:, cc, off:off + dw],
                                             start=(cc == 0), stop=(cc == NC - 1))
                        vst = sb3.tile([P, 512], BF16, tag="kvst")
                        nc.scalar.copy(out=vst[:, 0:dw], in_=ps[:, 0:dw])
                        nc.sync.dma_start(out=vv_view[:, t, off:off + dw],
                                          in_=vst[:, 0:dw])
                nc.gpsimd.collective_compute(
                    "AllGather", ALU.bypass, replica_groups=pair_groups,
                    ins=[kv_in.ap()], outs=[kv_out.ap()])
                wq = load_w(wq_all[l])
                for co in range(NC):
                    ps = psp.tile([P, TL], F32, tag="mm")
                    for cc in range(NC):
                        nc.tensor.matmul(ps[:, :], wq[:, cc, co * P:(co + 1) * P],
                                         hT[:, cc, :], start=(cc == 0),
                                         stop=(cc == NC - 1))
                    nc.scalar.copy(out=qT[:, co, :], in_=ps[:, :])

            # unpack gathered K/V into global order
            for g in range(NKT):
                r, slot = G2SLOT[g]
                kv_r = kv_out[r]
                ksrc = kv_r[0:KSZ].rearrange("(c p t) -> p c t", c=NC, p=P)
                nc.sync.dma_start(out=kT_full[:, :, g * P:(g + 1) * P],
                                  in_=ksrc[:, :, slot * P:(slot + 1) * P])
                vsrc = kv_r[KSZ:KVE].rearrange("(t p h d) -> t p h d",
                                               t=NBL, p=P, h=H)
                nc.sync.dma_start(out=v_full[:, g, :, 0:HD], in_=vsrc[slot])

            # ---- attention (k-major scores; sums via augmented V) ----
            with tc.tile_pool(name=f"ps_at{l}", bufs=3, space="PSUM") as psp, \
                 tc.tile_pool(name=f"ps_ctx{l}", bufs=2, space="PSUM") as psc:
                for h in range(H):
                    cc = (h * HD) // P
                    po = (h * HD) % P
                    ctx_ps = psc.tile([HD + 1, TL], F32, tag="ctx")
                    for kt in range(NKT):
                        qoff = (kt // 2) * P
                        NQ = TL - qoff
                        s_ps = psp.tile([P, TL], F32, tag="s")
                        nc.tensor.matmul(
                            s_ps[:, 0:NQ],
                            kT_full[po:po + HD, cc, kt * P:(kt + 1) * P],
                            qT[po:po + HD, cc, qoff:TL], start=True, stop=True)
                        nc.vector.tensor_tensor(out=s_ps[:, 0:P], in0=s_ps[:, 0:P],
                                                in1=maskT[:, kt, :], op=ALU.add)
                        pr = probs_pool.tile([P, TL], BF16, tag="probs")
                        nc.scalar.activation(out=pr[:, 0:NQ], in_=s_ps[:, 0:NQ],
                                             func=AF.Exp,
                                             scale=1.0 / math.sqrt(HD))
                        nc.tensor.matmul(ctx_ps[:, qoff:TL],
                                         v_full[:, kt, h, :], pr[:, 0:NQ],
                                         start=(kt == 0), stop=(kt == NKT - 1),
                                         skip_group_check=True)
                    # normalize columns by the sums row (row HD of ctx_ps)
                    rsum = stat.tile([P, TL], F32, tag="rsum")
                    nc.vector.reciprocal(out=rsum[HD:HD + 1, :],
                                         in_=ctx_ps[HD:HD + 1, :])
                    rb_ps = psp.tile([HD, TL], F32, tag="rb")
                    nc.tensor.matmul(rb_ps[:, :], ones_col[HD:HD + 1, :],
                                     rsum[HD:HD + 1, :], start=True, stop=True)
                    rb = sb3.tile([HD, TL], F32, tag="rbs")
                    nc.scalar.copy(out=rb[:, :], in_=rb_ps[:, :])
                    nc.vector.tensor_tensor(out=ctxT[po:po + HD, cc, :],
                                            in0=ctx_ps[0:HD, :], in1=rb[:, :],
                                            op=ALU.mult)

            # ---- output projection + residual ----
            with tc.tile_pool(name=f"ps_o{l}", bufs=3, space="PSUM") as psp:
                wo = load_w(wo_all[l])
                for t in range(NBL):
                    for off, dw in DHS:
                        ps = psp.tile([P, 512], F32, tag="mm2")
                        for cc2 in range(NC):
                            nc.tensor.matmul(ps[:, 0:dw], ctxT[:, cc2, t * P:(t + 1) * P],
                                             wo[:, cc2, off:off + dw],
                                             start=(cc2 == 0), stop=(cc2 == NC - 1))
                        nc.vector.tensor_tensor(
                            out=x[:, t, off:off + dw],
                            in0=x[:, t, off:off + dw], in1=ps[:, 0:dw],
                            op=ALU.add)

            # ---- MLP ----
            h2 = sb3.tile([P, NBL, D], BF16, tag="h_bf")
            with tc.tile_pool(name=f"ps_tp2{l}", bufs=2, space="PSUM") as psp:
                for t in range(NBL):
                    layernorm(x[:, t, :], h2[:, t, :])
                transpose_to(h2, hT, psp)

            wfc_src = wfc_all[l].rearrange("(c p) o -> p c o", p=P)
            wpr_src = wpr_all[l].rearrange("(g p) o -> p g o", p=P)
            for rnd in range(NROUND):
                with tc.tile_pool(name=f"ps_mlp{l}_{rnd}", bufs=3, space="PSUM") as psp:
                    uT = upool.tile([P, RM, TL], BF16, tag="uT",
                                    name=f"uT_{l}_{rnd}")
                    for mgl in range(RM // 4):
                        mg = rnd * (RM // 4) + mgl
                        wfc = wpool.tile([P, NC, 512], BF16, tag="wstream")
                        nc.sync.dma_start(out=wfc,
                                          in_=wfc_src[:, :, mg * 512:(mg + 1) * 512])
                        for m in range(4):
                            ps = psp.tile([P, TL], F32, tag="mm")
                            for cc in range(NC):
                                nc.tensor.matmul(ps[:, :], wfc[:, cc, m * P:(m + 1) * P],
                                                 hT[:, cc, :], start=(cc == 0),
                                                 stop=(cc == NC - 1))
                            if cfg.get("gelu") == "sigmoid":
                                sg = sb3.tile([P, TL], F32, tag="sg")
                                nc.scalar.activation(out=sg[:, :], in_=ps[:, :],
                                                     func=AF.Sigmoid, scale=1.702)
                                nc.vector.tensor_tensor(out=uT[:, mgl * 4 + m, :],
                                                        in0=ps[:, :], in1=sg[:, :],
                                                        op=ALU.mult)
                            else:
                                nc.scalar.activation(out=uT[:, mgl * 4 + m, :],
                                                     in_=ps[:, :], func=AF.Gelu)
                    for off, dw in DHS:
                        wpr = wpool.tile([P, RM, 512], BF16, tag="wstream")
                        nc.sync.dma_start(
                            out=wpr[:, :, 0:dw],
                            in_=wpr_src[:, rnd * RM:(rnd + 1) * RM, off:off + dw])
                        for t in range(NBL):
                            ps = psp.tile([P, 512], F32, tag="mm2")
                            for hh in range(RM):
                                nc.tensor.matmul(
                                    ps[:, 0:dw], uT[:, hh, t * P:(t + 1) * P],
                                    wpr[:, hh, 0:dw], start=(hh == 0),
                                    stop=(hh == RM - 1))
                            nc.vector.tensor_tensor(
                                out=x[:, t, off:off + dw],
                                in0=x[:, t, off:off + dw], in1=ps[:, 0:dw],
                                op=ALU.add)

        # ================= final LN + lm head =================
        xrow = lm1.tile([1, D], F32, tag="xrow")
        nc.gpsimd.dma_start(out=xrow[0:1, :], in_=x[P - 1:P, NBL - 1, :])
        xln = lm1.tile([P, D], F32, tag="xln")
        layernorm(xrow[0:1, :], xln[0:1, :], plo=0, n_rows=1)
        xl0 = lm1.tile([1, D], BF16, tag="xl0")
        nc.vector.tensor_copy(out=xl0[0:1, :], in_=xln[0:1, :])
        oh = stat.tile([1, B], BF16, tag="oh")
        nc.sync.dma_start(out=oh, in_=onehot_in.ap())
        contrib = lm1.tile([B, D], F32, tag="contrib")
        with tc.tile_pool(name="ps_lm", bufs=4, space="PSUM") as psp:
            for off, dw in DHS:
                ps = psp.tile([B, 512], F32, tag="lmps")
                nc.tensor.matmul(ps[:, 0:dw], oh[0:1, :],
                                 xl0[0:1, off:off + dw],
                                 start=True, stop=True)
                nc.scalar.copy(out=contrib[:, off:off + dw], in_=ps[:, 0:dw])
            nc.sync.dma_start(out=lm_in.ap(), in_=contrib)
            nc.gpsimd.collective_compute(
                "AllReduce", ALU.add, replica_groups=all_group,
                ins=[lm_in.ap()], outs=[lm_out.ap()])
            xl_all = lm1.tile([B, D], F32, tag="xl_all")
            nc.sync.dma_start(out=xl_all, in_=lm_out.ap())
            xl_bf = lm1.tile([B, D], BF16, tag="xl_bf")
            nc.vector.tensor_copy(out=xl_bf[:, :], in_=xl_all[:, :])
            xlT = lm1.tile([P, NC, B], BF16, tag="xlT")
            for cc in range(NC):
                tp = psp.tile([P, B], BF16, tag="lmtp")
                nc.tensor.transpose(out=tp[:, :], in_=xl_bf[:, cc * P:(cc + 1) * P],
                                    identity=ident[0:B, 0:B])
                nc.scalar.copy(out=xlT[:, cc, :], in_=tp[:, :])
            for vt in range(NVT):
                nv = min(512, VS - vt * 512)
                wvt = wpool.tile([P, NC, 512], BF16, tag="wstream")
                src = wteT_in.rearrange("(c p) v -> p c v", p=P)
                nc.sync.dma_start(out=wvt[:, :, 0:nv],
                                  in_=src[:, :, vt * 512:vt * 512 + nv])
                ps = psp.tile([B, 512], F32, tag="lmps")
                for cc in range(NC):
                    nc.tensor.matmul(ps[:, 0:nv], xlT[:, cc, :], wvt[:, cc, 0:nv],
                                     start=(cc == 0), stop=(cc == NC - 1))
                lmout = sb3.tile([B, 512], F32, tag="lmout")
                nc.scalar.copy(out=lmout[:, 0:nv], in_=ps[:, 0:nv])
                nc.sync.dma_start(out=logits_out.ap()[:, vt * 512:vt * 512 + nv],
                                  in_=lmout[:, 0:nv])

    nc.compile()
    return nc


# --------------------------------------------------------------------------
# host side
# --------------------------------------------------------------------------

def _bf16(a):
    return np.asarray(a, dtype=np.float32).astype(ml_dtypes.bfloat16)


def shard_inputs(cfg, inputs):
    c = derived(cfg)
    B, T, D, L, V, NCORES = cfg["B"], cfg["T"], cfg["D"], cfg["L"], cfg["V"], cfg["NCORES"]
    VS = c["VS"]
    idx = np.asarray(inputs["idx"]).astype(np.int32)
    wte = np.asarray(inputs["wte"], dtype=np.float32)
    wpe = np.asarray(inputs["wpe"], dtype=np.float32)
    ln1_g = np.asarray(inputs["ln1_g"], dtype=np.float32)
    ln2_g = np.asarray(inputs["ln2_g"], dtype=np.float32)
    lnf_g = np.asarray(inputs["lnf_g"], dtype=np.float32)

    # fold LN gammas into the consuming weights (exact rewrite)
    wq = _bf16(inputs["wq"] * ln1_g[:, :, None])
    wk = _bf16(inputs["wk"] * ln1_g[:, :, None])
    wv = _bf16(inputs["wv"] * ln1_g[:, :, None])
    wo = _bf16(inputs["wo"])
    wfc = _bf16(inputs["wfc"] * ln2_g[:, :, None])
    wpr = _bf16(inputs["wpr"])
    wte_emb = _bf16(wte)
    wteT = wte.T * lnf_g[:, None]            # [D, V]
    wteT_pad = np.zeros((D, VS * NCORES), dtype=np.float32)
    wteT_pad[:, :V] = wteT
    wteT_pad = _bf16(wteT_pad)

    in_maps = []
    for core in range(NCORES):
        pair, rank = divmod(core, 2)
        G = G_EVEN if rank == 0 else G_ODD
        rows = np.concatenate([np.arange(g * P, (g + 1) * P) for g in G])
        # k-major boundary masks: for k-tile kt the first valid q block is kt//2
        maskT = np.zeros((c["NKT"], P, P), dtype=np.float32)
        for kt in range(c["NKT"]):
            jb = kt // 2
            k_abs = kt * P + np.arange(P)[:, None]
            q_abs = G[jb] * P + np.arange(P)[None, :]
            maskT[kt] = np.where(k_abs <= q_abs, 0.0, NEG)
        onehot = np.zeros((1, B), dtype=np.float32)
        if rank == 0:                         # owns global block 7 (last token)
            onehot[0, pair] = 1.0
        in_maps.append({
            "x_idx": idx[pair][rows].reshape(c["NBL"], P),
            "wpe_loc": wpe[rows].reshape(c["NBL"], P, D),
            "wte_emb": wte_emb,
            "wq_all": wq, "wk_all": wk, "wv_all": wv, "wo_all": wo,
            "wfc_all": wfc, "wpr_all": wpr,
            "maskT": maskT,
            "onehot": _bf16(onehot),
            "wteT_loc": np.ascontiguousarray(wteT_pad[:, core * VS:(core + 1) * VS]),
        })
    return in_maps


def assemble(cfg, results):
    c = derived(cfg)
    V, VS = cfg["V"], c["VS"]
    logits = np.concatenate([r["logits_loc"] for r in results], axis=1)
    return np.ascontiguousarray(logits[:, :V]).astype(np.float32)


def kernel(**inputs):
    from concourse import bass_utils
    cfg = full_cfg()
    nc = build_program(cfg)
    in_maps = shard_inputs(cfg, inputs)
    res = bass_utils.run_bass_kernel_spmd(nc, in_maps,
                                          core_ids=list(range(cfg["NCORES"])))
    return assemble(cfg, res.results)


# revision 17
# speedup vs baseline: 16.8013x; 16.8013x over previous
"""GPT-2-like transformer forward on 8 Trainium2 NeuronCores.

Sharding: batch (4) -> core pairs; within a pair, sequence-parallel split of
the 8 query blocks of 128 (zigzag {0,3,4,7}/{1,2,5,6} for causal balance).
Per layer: one K/V AllGather inside each pair. lm_head vocab-sharded 8 ways
(tiny AllReduce of the 4 final-position vectors first). Matmuls in bf16 with
fp32 accumulation; residual stream fp32. LN gammas are folded into the
following weight matrices on the host (exact rewrite).
"""

import math
from contextlib import ExitStack

import numpy as np
import ml_dtypes

import concourse.bass as bass
import concourse.mybir as mybir
import concourse.tile as tile
from concourse import bacc
from concourse.masks import make_identity

BF16 = mybir.dt.bfloat16
F32 = mybir.dt.float32
I32 = mybir.dt.int32
AF = mybir.ActivationFunctionType
ALU = mybir.AluOpType

P = 128
NEG = -1.0e9

G_EVEN = [0, 3, 4, 7]
G_ODD = [1, 2, 5, 6]
# global k-tile g -> (rank in pair, slot in that rank's shard)
G2SLOT = {}
for _s, _g in enumerate(G_EVEN):
    G2SLOT[_g] = (0, _s)
for _s, _g in enumerate(G_ODD):
    G2SLOT[_g] = (1, _s)


def full_cfg():
    return dict(B=4, T=1024, D=1024, H=16, HD=64, L=12, DFF=4096, V=50257,
                NCORES=8, EPS=1e-5)


def derived(cfg):
    d = dict(cfg)
    d["NC"] = cfg["D"] // P              # dim chunks
    d["NBL"] = 4                          # local q blocks per core
    d["NKT"] = 8                          # global k tiles
    d["TL"] = d["NBL"] * P                # local tokens (512)
    d["NM"] = cfg["DFF"] // P             # mlp hidden chunks
    d["NMG"] = cfg["DFF"] // 512          # mlp hidden groups of 512
    d["VS"] = math.ceil(cfg["V"] / cfg["NCORES"])   # vocab shard
    d["NPAIR"] = cfg["NCORES"] // 2
    d["KSZ"] = d["NC"] * P * d["TL"]      # k elements in kv slab
    d["VSZ"] = d["NBL"] * P * cfg["D"]    # v elements in kv slab
    d["KVE"] = d["KSZ"] + d["VSZ"]
    return d


# --------------------------------------------------------------------------
# device program
# --------------------------------------------------------------------------

def build_program(cfg):
    c = derived(cfg)
    B, D, H, HD, L, DFF, NCORES = (c["B"], c["D"], c["H"], c["HD"], c["L"],
                                   c["DFF"], c["NCORES"])
    NC, NBL, NKT, TL, NM, NMG, VS = (c["NC"], c["NBL"], c["NKT"], c["TL"],
                                     c["NM"], c["NMG"], c["VS"])
    KSZ, KVE, VSZ = c["KSZ"], c["KVE"], c["VSZ"]
    EPS = c["EPS"]
    NVT = math.ceil(VS / 512)             # vocab tiles per core
    DHS = [(i * 512, min(512, D - i * 512)) for i in range((D + 511) // 512)]
    HGRP = min(8, NM)                     # wpr hidden chunks per dma group
    NHG = (NM + HGRP - 1) // HGRP

    nc = bacc.Bacc("TRN2", target_bir_lowering=False, debug=False,
                   num_devices=NCORES)

    # ---- I/O ----
    idx_in = nc.declare_dram_parameter("x_idx", [NBL, P], I32, isOutput=False)
    wpe_in = nc.declare_dram_parameter("wpe_loc", [NBL, P, D], F32, isOutput=False)
    wte_emb = nc.declare_dram_parameter("wte_emb", [cfg["V"], D], BF16, isOutput=False)
    wq_all = nc.declare_dram_parameter("wq_all", [L, D, D], BF16, isOutput=False)
    wk_all = nc.declare_dram_parameter("wk_all", [L, D, D], BF16, isOutput=False)
    wv_all = nc.declare_dram_parameter("wv_all", [L, D, D], BF16, isOutput=False)
    wo_all = nc.declare_dram_parameter("wo_all", [L, D, D], BF16, isOutput=False)
    wfc_all = nc.declare_dram_parameter("wfc_all", [L, D, DFF], BF16, isOutput=False)
    wpr_all = nc.declare_dram_parameter("wpr_all", [L, DFF, D], BF16, isOutput=False)
    maskT_in = nc.declare_dram_parameter("maskT", [NKT, P, P], F32, isOutput=False)
    onehot_in = nc.declare_dram_parameter("onehot", [1, B], BF16, isOutput=False)
    wteT_in = nc.declare_dram_parameter("wteT_loc", [D, VS], BF16, isOutput=False)
    logits_out = nc.declare_dram_parameter("logits_loc", [B, VS], F32, isOutput=True)

    # ---- internal DRAM for collectives ----
    k_in = nc.dram_tensor("k_in", [KSZ], BF16)
    k_out = nc.dram_tensor("k_out", [2, KSZ], BF16)
    v_in = nc.dram_tensor("v_in", [VSZ], BF16)
    v_out = nc.dram_tensor("v_out", [2, VSZ], BF16)
    lm_in = nc.dram_tensor("lm_in", [B, D], F32)
    lm_out = nc.dram_tensor("lm_out", [B, D], F32,
                            addr_space="Shared" if NCORES > 4 else "Local")

    pair_groups = [[2 * p, 2 * p + 1] for p in range(c["NPAIR"])]
    all_group = [list(range(NCORES))]

    with tile.TileContext(nc) as tc, ExitStack() as ctx:
        persist = ctx.enter_context(tc.tile_pool(name="persist", bufs=1))
        wpool = ctx.enter_context(tc.tile_pool(name="wpool", bufs=2))
        sb3 = ctx.enter_context(tc.tile_pool(name="sb3", bufs=2))
        probs_pool = ctx.enter_context(tc.tile_pool(name="probs_pool", bufs=4))
        stat = ctx.enter_context(tc.tile_pool(name="stat", bufs=4))
        lm1 = ctx.enter_context(tc.tile_pool(name="lm1", bufs=1))

        # persistent state
        x = persist.tile([P, NBL, D], F32, tag="x")
        hT = persist.tile([P, NC, TL], BF16, tag="hT")
        qT = persist.tile([P, NC, TL], BF16, tag="qT")
        kT_full = persist.tile([P, NC, NKT * P], BF16, tag="kT_full")
        v_full = persist.tile([P, NKT, H, HD + 1], BF16, tag="v_full")
        ctxT = persist.tile([P, NC, TL], BF16, tag="ctxT")
        RM = min(8, NM)
        NROUND = NM // RM
        upool = ctx.enter_context(tc.tile_pool(name="upool", bufs=2))

        maskT = persist.tile([P, NKT, P], F32, tag="maskT")
        ident = persist.tile([P, P], BF16, tag="ident")
        ones_col = persist.tile([P, HD], F32, tag="ones_col")
        eps_t = persist.tile([P, 1], F32, tag="eps")

        make_identity(nc, ident)
        nc.vector.memset(ones_col, 1.0)
        nc.vector.memset(eps_t, EPS)
        # ones column of the augmented V (sums row of PV output)
        nc.vector.memset(v_full[:, :, :, HD:HD + 1], 1.0)
        nc.sync.dma_start(out=maskT, in_=maskT_in.rearrange("k p q -> p k q"))

        # ---- embeddings ----
        for t in range(NBL):
            idx_t = stat.tile([P, 1], I32, tag="idx")
            nc.sync.dma_start(out=idx_t, in_=idx_in[t, :].rearrange("(p o) -> p o", o=1))
            emb = sb3.tile([P, D], BF16, tag="h_bf")
            nc.gpsimd.indirect_dma_start(
                out=emb[:, :], out_offset=None, in_=wte_emb[:, :],
                in_offset=bass.IndirectOffsetOnAxis(ap=idx_t[:, 0:1], axis=0))
            wpe_t = sb3.tile([P, D], F32, tag="h_bf")
            nc.sync.dma_start(out=wpe_t, in_=wpe_in[t])
            nc.vector.tensor_tensor(out=x[:, t, :], in0=wpe_t[:, :],
                                    in1=emb[:, :], op=ALU.add)

        def layernorm(src_ap, dst_ap, plo=0, n_rows=P):
            # src [n_rows, D] fp32 -> dst [n_rows, D] bf16 (gamma folded into W)
            phi = plo + n_rows
            ngrp = max(1, D // 512)
            st = stat.tile([P, ngrp, 6], F32, tag="bnst")
            grp = src_ap.rearrange("p (g d) -> p g d", g=ngrp)
            for g in range(ngrp):
                nc.vector.bn_stats(out=st[plo:phi, g, :], in_=grp[:, g, :])
            mv = stat.tile([P, 2], F32, tag="bnmv")
            nc.vector.bn_aggr(out=mv[plo:phi], in_=st[plo:phi])
            rs = stat.tile([P, 1], F32, tag="bnrs")
            nc.scalar.activation(out=rs[plo:phi], in_=mv[plo:phi, 1:2],
                                 func=AF.Sqrt, bias=eps_t[plo:phi])
            nc.vector.reciprocal(out=rs[plo:phi], in_=rs[plo:phi])
            nc.vector.tensor_scalar(out=dst_ap, in0=src_ap,
                                    scalar1=mv[plo:phi, 0:1], scalar2=rs[plo:phi],
                                    op0=ALU.subtract, op1=ALU.mult)

        def transpose_to(h_bf, dstT, psp):
            # h_bf [P, NBL, D] bf16 token-major -> dstT [P, NC, TL] feature-major
            for t in range(NBL):
                for cc in range(NC):
                    tp = psp.tile([P, P], BF16, tag="tp")
                    nc.tensor.transpose(out=tp[:, :], in_=h_bf[:, t, cc * P:(cc + 1) * P],
                                        identity=ident[:, :])
                    nc.scalar.copy(out=dstT[:, cc, t * P:(t + 1) * P], in_=tp[:, :])

        def load_w(dram_l):
            w = wpool.tile([P, NC, D], BF16, tag="w4")
            nc.sync.dma_start(out=w, in_=dram_l.rearrange("(c p) o -> p c o", p=P))
            return w

        # ================= layers =================
        for l in range(L):
            h_bf = sb3.tile([P, NBL, D], BF16, tag="h_bf")
            with tc.tile_pool(name=f"ps_tp{l}", bufs=2, space="PSUM") as psp:
                for t in range(NBL):
                    layernorm(x[:, t, :], h_bf[:, t, :])
                transpose_to(h_bf, hT, psp)

            # ---- K, V, Q projections ----
            with tc.tile_pool(name=f"ps_kvq{l}", bufs=3, space="PSUM") as psp:
                vk_view = k_in.ap().rearrange("(c p t) -> p c t", c=NC, p=P)
                vv_view = v_in.ap().rearrange("(t p d) -> p t d", t=NBL, p=P)
                wk = load_w(wk_all[l])
                for co in range(NC):
                    ps = psp.tile([P, TL], F32, tag="mm")
                    for cc in range(NC):
                        nc.tensor.matmul(ps[:, :], wk[:, cc, co * P:(co + 1) * P],
                                         hT[:, cc, :], start=(cc == 0),
                                         stop=(cc == NC - 1))
                    kst = sb3.tile([P, TL], BF16, tag="kvst")
                    nc.scalar.copy(out=kst[:, :], in_=ps[:, :])
                    nc.sync.dma_start(out=vk_view[:, co, :], in_=kst)
                nc.gpsimd.collective_compute(
                    "AllGather", ALU.bypass, replica_groups=pair_groups,
                    ins=[k_in.ap()], outs=[k_out.ap()])
                wv = load_w(wv_all[l])
                for t in range(NBL):
                    for off, dw in DHS:
                        ps = psp.tile([P, 512], F32, tag="mm2")
                        for cc in range(NC):
                            nc.tensor.matmul(ps[:, 0:dw], hT[:, cc, t * P:(t + 1) * P],
                                             wv[:, cc, off:off + dw],
                                             start=(cc == 0), stop=(cc == NC - 1))
                        vst = sb3.tile([P, 512], BF16, tag="kvst")
                        nc.scalar.copy(out=vst[:, 0:dw], in_=ps[:, 0:dw])
                        nc.sync.dma_start(out=vv_view[:, t, off:off + dw],
                                          in_=vst[:, 0:dw])
                nc.gpsimd.collective_compute(
                    "AllGather", ALU.bypass, replica_groups=pair_groups,
                    ins=[v_in.ap()], outs=[v_out.ap()])
                wq = load_w(wq_all[l])
                for co in range(NC):
                    ps = psp.tile([P, TL], F32, tag="mm")
                    for cc in range(NC):
                        nc.tensor.matmul(ps[:, :], wq[:, cc, co * P:(co + 1) * P],
                                         hT[:, cc, :], start=(cc == 0),
                                         stop=(cc == NC - 1))
                    nc.scalar.copy(out=qT[:, co, :], in_=ps[:, :])

            # unpack gathered K/V into global order
            for g in range(NKT):
                r, slot = G2SLOT[g]
                ksrc = k_out[r].rearrange("(c p t) -> p c t", c=NC, p=P)
                nc.sync.dma_start(out=kT_full[:, :, g * P:(g + 1) * P],
                                  in_=ksrc[:, :, slot * P:(slot + 1) * P])
            for g in range(NKT):
                r, slot = G2SLOT[g]
                vsrc = v_out[r].rearrange("(t p h d) -> t p h d",
                                          t=NBL, p=P, h=H)
                nc.sync.dma_start(out=v_full[:, g, :, 0:HD], in_=vsrc[slot])

            # ---- attention (k-major scores; sums via augmented V) ----
            with tc.tile_pool(name=f"ps_at{l}", bufs=4, space="PSUM") as psp, \
                 tc.tile_pool(name=f"ps_ctx{l}", bufs=2, space="PSUM") as psc:
                for h in range(H):
                    cc = (h * HD) // P
                    po = (h * HD) % P
                    ctx_ps = psc.tile([HD + 1, TL], F32, tag="ctx")
                    for kt in range(NKT):
                        qoff = (kt // 2) * P
                        NQ = TL - qoff
                        s_ps = psp.tile([P, TL], F32, tag="s")
                        nc.tensor.matmul(
                            s_ps[:, 0:NQ],
                            kT_full[po:po + HD, cc, kt * P:(kt + 1) * P],
                            qT[po:po + HD, cc, qoff:TL], start=True, stop=True)
                        nc.vector.tensor_tensor(out=s_ps[:, 0:P], in0=s_ps[:, 0:P],
                                                in1=maskT[:, kt, :], op=ALU.add)
                        pr = probs_pool.tile([P, TL], BF16, tag="probs")
                        nc.scalar.activation(out=pr[:, 0:NQ], in_=s_ps[:, 0:NQ],
                                             func=AF.Exp,
                                             scale=1.0 / math.sqrt(HD))
                        nc.tensor.matmul(ctx_ps[:, qoff:TL],
                                         v_full[:, kt, h, :], pr[:, 0:NQ],
                                         start=(kt == 0), stop=(kt == NKT - 1),
                                         skip_group_check=True)
                    # normalize columns by the sums row (row HD of ctx_ps)
                    rsum = stat.tile([P, TL], F32, tag="rsum")
                    nc.vector.reciprocal(out=rsum[HD:HD + 1, :],
                                         in_=ctx_ps[HD:HD + 1, :])
                    rb_ps = psc.tile([HD, TL], F32, tag="rb")
                    nc.tensor.matmul(rb_ps[:, :], ones_col[HD:HD + 1, :],
                                     rsum[HD:HD + 1, :], start=True, stop=True)
                    rb = sb3.tile([HD, TL], F32, tag="rbs")
                    nc.scalar.copy(out=rb[:, :], in_=rb_ps[:, :])
                    nc.vector.tensor_tensor(out=ctxT[po:po + HD, cc, :],
                                            in0=ctx_ps[0:HD, :], in1=rb[:, :],
                                            op=ALU.mult)

            # ---- output projection + residual ----
            with tc.tile_pool(name=f"ps_o{l}", bufs=3, space="PSUM") as psp:
                wo = load_w(wo_all[l])
                for t in range(NBL):
                    for off, dw in DHS:
                        ps = psp.tile([P, 512], F32, tag="mm2")
                        for cc2 in range(NC):
                            nc.tensor.matmul(ps[:, 0:dw], ctxT[:, cc2, t * P:(t + 1) * P],
                                             wo[:, cc2, off:off + dw],
                                             start=(cc2 == 0), stop=(cc2 == NC - 1))
                        nc.vector.tensor_tensor(
                            out=x[:, t, off:off + dw],
                            in0=x[:, t, off:off + dw], in1=ps[:, 0:dw],
                            op=ALU.add)

            # ---- MLP ----
            h2 = sb3.tile([P, NBL, D], BF16, tag="h_bf")
            with tc.tile_pool(name=f"ps_tp2{l}", bufs=2, space="PSUM") as psp:
                for t in range(NBL):
                    layernorm(x[:, t, :], h2[:, t, :])
                transpose_to(h2, hT, psp)

            wfc_src = wfc_all[l].rearrange("(c p) o -> p c o", p=P)
            wpr_src = wpr_all[l].rearrange("(g p) o -> p g o", p=P)
            for rnd in range(NROUND):
                with tc.tile_pool(name=f"ps_mlp{l}_{rnd}", bufs=3, space="PSUM") as psp:
                    uT = upool.tile([P, RM, TL], BF16, tag="uT",
                                    name=f"uT_{l}_{rnd}")
                    for mgl in range(RM // 4):
                        mg = rnd * (RM // 4) + mgl
                        wfc = wpool.tile([P, NC, 512], BF16, tag="wstream")
                        nc.sync.dma_start(out=wfc,
                                          in_=wfc_src[:, :, mg * 512:(mg + 1) * 512])
                        for m in range(4):
                            ps = psp.tile([P, TL], F32, tag="mm")
                            for cc in range(NC):
                                nc.tensor.matmul(ps[:, :], wfc[:, cc, m * P:(m + 1) * P],
                                                 hT[:, cc, :], start=(cc == 0),
                                                 stop=(cc == NC - 1))
                            if cfg.get("gelu") == "sigmoid":
                                sg = sb3.tile([P, TL], F32, tag="sg")
                                nc.scalar.activation(out=sg[:, :], in_=ps[:, :],
                                                     func=AF.Sigmoid, scale=1.702)
                                nc.vector.tensor_tensor(out=uT[:, mgl * 4 + m, :],
                                                        in0=ps[:, :], in1=sg[:, :],
                                                        op=ALU.mult)
                            else:
                                nc.scalar.activation(out=uT[:, mgl * 4 + m, :],
                                                     in_=ps[:, :], func=AF.Gelu)
                    for off, dw in DHS:
                        wpr = wpool.tile([P, RM, 512], BF16, tag="wstream")
                        nc.sync.dma_start(
                            out=wpr[:, :, 0:dw],
                            in_=wpr_src[:, rnd * RM:(rnd + 1) * RM, off:off + dw])
                        for t in range(NBL):
                            ps = psp.tile([P, 512], F32, tag="mm2")
                            for hh in range(RM):
                                nc.tensor.matmul(
                                    ps[:, 0:dw], uT[:, hh, t * P:(t + 1) * P],
                                    wpr[:, hh, 0:dw], start=(hh == 0),
                                    stop=(hh == RM - 1))
                            nc.vector.tensor_tensor(
                                out=x[:, t, off:off + dw],
                                in0=x[:, t, off:off + dw], in1=ps[:, 0:dw],
                                op=ALU.add)

        # ================= final LN + lm head =================
        xrow = lm1.tile([1, D], F32, tag="xrow")
        nc.gpsimd.dma_start(out=xrow[0:1, :], in_=x[P - 1:P, NBL - 1, :])
        xln = lm1.tile([P, D], F32, tag="xln")
        layernorm(xrow[0:1, :], xln[0:1, :], plo=0, n_rows=1)
        xl0 = lm1.tile([1, D], BF16, tag="xl0")
        nc.vector.tensor_copy(out=xl0[0:1, :], in_=xln[0:1, :])
        oh = stat.tile([1, B], BF16, tag="oh")
        nc.sync.dma_start(out=oh, in_=onehot_in.ap())
        contrib = lm1.tile([B, D], F32, tag="contrib")
        with tc.tile_pool(name="ps_lm", bufs=4, space="PSUM") as psp:
            for off, dw in DHS:
                ps = psp.tile([B, 512], F32, tag="lmps")
                nc.tensor.matmul(ps[:, 0:dw], oh[0:1, :],
                                 xl0[0:1, off:off + dw],
                                 start=True, stop=True)
                nc.scalar.copy(out=contrib[:, off:off + dw], in_=ps[:, 0:dw])
            nc.sync.dma_start(out=lm_in.ap(), in_=contrib)
            nc.gpsimd.collective_compute(
                "AllReduce", ALU.add, replica_groups=all_group,
                ins=[lm_in.ap()], outs=[lm_out.ap()])
            xl_all = lm1.tile([B, D], F32, tag="xl_all")
            nc.sync.dma_start(out=xl_all, in_=lm_out.ap())
            xl_bf = lm1.tile([B, D], BF16, tag="xl_bf")
            nc.vector.tensor_copy(out=xl_bf[:, :], in_=xl_all[:, :])
            xlT = lm1.tile([P, NC, B], BF16, tag="xlT")
            for cc in range(NC):
                tp = psp.tile([P, B], BF16, tag="lmtp")
                nc.tensor.transpose(out=tp[:, :], in_=xl_bf[:, cc * P:(cc + 1) * P],
                                    identity=ident[0:B, 0:B])
                nc.scalar.copy(out=xlT[:, cc, :], in_=tp[:, :])
            for vt in range(NVT):
                nv = min(512, VS - vt * 512)
                wvt = wpool.tile([P, NC, 512], BF16, tag="wstream")
                src = wteT_in.rearrange("(c p) v -> p c v", p=P)
                nc.sync.dma_start(out=wvt[:, :, 0:nv],
                                  in_=src[:, :, vt * 512:vt * 512 + nv])
                ps = psp.tile([B, 512], F32, tag="lmps")
                for cc in range(NC):
                    nc.tensor.matmul(ps[:, 0:nv], xlT[:, cc, :], wvt[:, cc, 0:nv],
                                     start=(cc == 0), stop=(cc == NC - 1))
                lmout = sb3.tile([B, 512], F32, tag="lmout")
                nc.scalar.copy(out=lmout[:, 0:nv], in_=ps[:, 0:nv])
                nc.sync.dma_start(out=logits_out.ap()[:, vt * 512:vt * 512 + nv],
                                  in_=lmout[:, 0:nv])

    nc.compile()
    return nc


# --------------------------------------------------------------------------
# host side
# --------------------------------------------------------------------------

def _bf16(a):
    return np.asarray(a, dtype=np.float32).astype(ml_dtypes.bfloat16)


def shard_inputs(cfg, inputs):
    c = derived(cfg)
    B, T, D, L, V, NCORES = cfg["B"], cfg["T"], cfg["D"], cfg["L"], cfg["V"], cfg["NCORES"]
    VS = c["VS"]
    idx = np.asarray(inputs["idx"]).astype(np.int32)
    wte = np.asarray(inputs["wte"], dtype=np.float32)
    wpe = np.asarray(inputs["wpe"], dtype=np.float32)
    ln1_g = np.asarray(inputs["ln1_g"], dtype=np.float32)
    ln2_g = np.asarray(inputs["ln2_g"], dtype=np.float32)
    lnf_g = np.asarray(inputs["lnf_g"], dtype=np.float32)

    # fold LN gammas into the consuming weights (exact rewrite)
    wq = _bf16(inputs["wq"] * ln1_g[:, :, None])
    wk = _bf16(inputs["wk"] * ln1_g[:, :, None])
    wv = _bf16(inputs["wv"] * ln1_g[:, :, None])
    wo = _bf16(inputs["wo"])
    wfc = _bf16(inputs["wfc"] * ln2_g[:, :, None])
    wpr = _bf16(inputs["wpr"])
    wte_emb = _bf16(wte)
    wteT = wte.T * lnf_g[:, None]            # [D, V]
    wteT_pad = np.zeros((D, VS * NCORES), dtype=np.float32)
    wteT_pad[:, :V] = wteT
    wteT_pad = _bf16(wteT_pad)

    in_maps = []
    for core in range(NCORES):
        pair, rank = divmod(core, 2)
        G = G_EVEN if rank == 0 else G_ODD
        rows = np.concatenate([np.arange(g * P, (g + 1) * P) for g in G])
        # k-major boundary masks: for k-tile kt the first valid q block is kt//2
        maskT = np.zeros((c["NKT"], P, P), dtype=np.float32)
        for kt in range(c["NKT"]):
            jb = kt // 2
            k_abs = kt * P + np.arange(P)[:, None]
            q_abs = G[jb] * P + np.arange(P)[None, :]
            maskT[kt] = np.where(k_abs <= q_abs, 0.0, NEG)
        onehot = np.zeros((1, B), dtype=np.float32)
        if rank == 0:                         # owns global block 7 (last token)
            onehot[0, pair] = 1.0
        in_maps.append({
            "x_idx": idx[pair][rows].reshape(c["NBL"], P),
            "wpe_loc": wpe[rows].reshape(c["NBL"], P, D),
            "wte_emb": wte_emb,
            "wq_all": wq, "wk_all": wk, "wv_all": wv, "wo_all": wo,
            "wfc_all": wfc, "wpr_all": wpr,
            "maskT": maskT,
            "onehot": _bf16(onehot),
            "wteT_loc": np.ascontiguousarray(wteT_pad[:, core * VS:(core + 1) * VS]),
        })
    return in_maps


def assemble(cfg, results):
    c = derived(cfg)
    V, VS = cfg["V"], c["VS"]
    logits = np.concatenate([r["logits_loc"] for r in results], axis=1)
    return np.ascontiguousarray(logits[:, :V]).astype(np.float32)


def kernel(**inputs):
    from concourse import bass_utils
    cfg = full_cfg()
    nc = build_program(cfg)
    in_maps = shard_inputs(cfg, inputs)
    res = bass_utils.run_bass_kernel_spmd(nc, in_maps,
                                          core_ids=list(range(cfg["NCORES"])))
    return assemble(cfg, res.results)


# revision 20
# speedup vs baseline: 16.8384x; 1.0022x over previous
"""GPT-2-like transformer forward on 8 Trainium2 NeuronCores.

Sharding: batch (4) -> core pairs; within a pair, sequence-parallel split of
the 8 query blocks of 128 (zigzag {0,3,4,7}/{1,2,5,6} for causal balance).
Per layer: one K/V AllGather inside each pair. lm_head vocab-sharded 8 ways
(tiny AllReduce of the 4 final-position vectors first). Matmuls in bf16 with
fp32 accumulation; residual stream fp32. LN gammas are folded into the
following weight matrices on the host (exact rewrite).
"""

import math
from contextlib import ExitStack

import numpy as np
import ml_dtypes

import concourse.bass as bass
import concourse.mybir as mybir
import concourse.tile as tile
from concourse import bacc
from concourse.masks import make_identity

BF16 = mybir.dt.bfloat16
F32 = mybir.dt.float32
I32 = mybir.dt.int32
AF = mybir.ActivationFunctionType
ALU = mybir.AluOpType

P = 128
NEG = -1.0e9

G_EVEN = [0, 3, 4, 7]
G_ODD = [1, 2, 5, 6]
# global k-tile g -> (rank in pair, slot in that rank's shard)
G2SLOT = {}
for _s, _g in enumerate(G_EVEN):
    G2SLOT[_g] = (0, _s)
for _s, _g in enumerate(G_ODD):
    G2SLOT[_g] = (1, _s)


def full_cfg():
    return dict(B=4, T=1024, D=1024, H=16, HD=64, L=12, DFF=4096, V=50257,
                NCORES=8, EPS=1e-5)


def derived(cfg):
    d = dict(cfg)
    d["NC"] = cfg["D"] // P              # dim chunks
    d["NBL"] = 4                          # local q blocks per core
    d["NKT"] = 8                          # global k tiles
    d["TL"] = d["NBL"] * P                # local tokens (512)
    d["NM"] = cfg["DFF"] // P             # mlp hidden chunks
    d["NMG"] = cfg["DFF"] // 512          # mlp hidden groups of 512
    d["VS"] = math.ceil(cfg["V"] / cfg["NCORES"])   # vocab shard
    d["NPAIR"] = cfg["NCORES"] // 2
    d["KSZ"] = d["NC"] * P * d["TL"]      # k elements in kv slab
    d["VSZ"] = d["NBL"] * P * cfg["D"]    # v elements in kv slab
    d["KVE"] = d["KSZ"] + d["VSZ"]
    return d


# --------------------------------------------------------------------------
# device program
# --------------------------------------------------------------------------

def build_program(cfg):
    c = derived(cfg)
    B, D, H, HD, L, DFF, NCORES = (c["B"], c["D"], c["H"], c["HD"], c["L"],
                                   c["DFF"], c["NCORES"])
    NC, NBL, NKT, TL, NM, NMG, VS = (c["NC"], c["NBL"], c["NKT"], c["TL"],
                                     c["NM"], c["NMG"], c["VS"])
    KSZ, KVE, VSZ = c["KSZ"], c["KVE"], c["VSZ"]
    EPS = c["EPS"]
    NVT = math.ceil(VS / 512)             # vocab tiles per core
    DHS = [(i * 512, min(512, D - i * 512)) for i in range((D + 511) // 512)]
    HGRP = min(8, NM)                     # wpr hidden chunks per dma group
    NHG = (NM + HGRP - 1) // HGRP

    nc = bacc.Bacc("TRN2", target_bir_lowering=False, debug=False,
                   num_devices=NCORES)

    # ---- I/O ----
    idx_in = nc.declare_dram_parameter("x_idx", [NBL, P], I32, isOutput=False)
    wpe_in = nc.declare_dram_parameter("wpe_loc", [NBL, P, D], F32, isOutput=False)
    wte_emb = nc.declare_dram_parameter("wte_emb", [cfg["V"], D], BF16, isOutput=False)
    wq_all = nc.declare_dram_parameter("wq_all", [L, D, D], BF16, isOutput=False)
    wk_all = nc.declare_dram_parameter("wk_all", [L, D, D], BF16, isOutput=False)
    wv_all = nc.declare_dram_parameter("wv_all", [L, D, D], BF16, isOutput=False)
    wo_all = nc.declare_dram_parameter("wo_all", [L, D, D], BF16, isOutput=False)
    wfc_all = nc.declare_dram_parameter("wfc_all", [L, D, DFF], BF16, isOutput=False)
    wpr_all = nc.declare_dram_parameter("wpr_all", [L, DFF, D], BF16, isOutput=False)
    maskT_in = nc.declare_dram_parameter("maskT", [NKT, P, P], F32, isOutput=False)
    onehot_in = nc.declare_dram_parameter("onehot", [1, B], BF16, isOutput=False)
    wteT_in = nc.declare_dram_parameter("wteT_loc", [D, VS], BF16, isOutput=False)
    logits_out = nc.declare_dram_parameter("logits_loc", [B, VS], F32, isOutput=True)

    # ---- internal DRAM for collectives ----
    k_in = nc.dram_tensor("k_in", [KSZ], BF16)
    k_out = nc.dram_tensor("k_out", [2, KSZ], BF16)
    v_in = nc.dram_tensor("v_in", [VSZ], BF16)
    v_out = nc.dram_tensor("v_out", [2, VSZ], BF16)
    lm_in = nc.dram_tensor("lm_in", [B, D], F32)
    lm_out = nc.dram_tensor("lm_out", [B, D], F32,
                            addr_space="Shared" if NCORES > 4 else "Local")

    pair_groups = [[2 * p, 2 * p + 1] for p in range(c["NPAIR"])]
    all_group = [list(range(NCORES))]

    with tile.TileContext(nc) as tc, ExitStack() as ctx:
        persist = ctx.enter_context(tc.tile_pool(name="persist", bufs=1))
        wpool = ctx.enter_context(tc.tile_pool(name="wpool", bufs=2))
        sb3 = ctx.enter_context(tc.tile_pool(name="sb3", bufs=2))
        probs_pool = ctx.enter_context(tc.tile_pool(name="probs_pool", bufs=6))
        stat = ctx.enter_context(tc.tile_pool(name="stat", bufs=4))
        lm1 = ctx.enter_context(tc.tile_pool(name="lm1", bufs=1))

        # persistent state
        x = persist.tile([P, NBL, D], F32, tag="x")
        hT = persist.tile([P, NC, TL], BF16, tag="hT")
        qT = persist.tile([P, NC, TL], BF16, tag="qT")
        kT_full = persist.tile([P, NC, NKT * P], BF16, tag="kT_full")
        v_full = persist.tile([P, NKT, H, HD + 1], BF16, tag="v_full")
        ctxT = persist.tile([P, NC, TL], BF16, tag="ctxT")
        RM = min(8, NM)
        NROUND = NM // RM
        upool = ctx.enter_context(tc.tile_pool(name="upool", bufs=2))

        maskT = persist.tile([P, NKT, P], F32, tag="maskT")
        ident = persist.tile([P, P], BF16, tag="ident")
        ones_col = persist.tile([P, HD], F32, tag="ones_col")
        eps_t = persist.tile([P, 1], F32, tag="eps")

        make_identity(nc, ident)
        nc.vector.memset(ones_col, 1.0)
        nc.vector.memset(eps_t, EPS)
        # ones column of the augmented V (sums row of PV output)
        nc.vector.memset(v_full[:, :, :, HD:HD + 1], 1.0)
        nc.sync.dma_start(out=maskT, in_=maskT_in.rearrange("k p q -> p k q"))

        # ---- embeddings ----
        for t in range(NBL):
            idx_t = stat.tile([P, 1], I32, tag="idx")
            nc.sync.dma_start(out=idx_t, in_=idx_in[t, :].rearrange("(p o) -> p o", o=1))
            emb = sb3.tile([P, D], BF16, tag="h_bf")
            nc.gpsimd.indirect_dma_start(
                out=emb[:, :], out_offset=None, in_=wte_emb[:, :],
                in_offset=bass.IndirectOffsetOnAxis(ap=idx_t[:, 0:1], axis=0))
            wpe_t = sb3.tile([P, D], F32, tag="h_bf")
            nc.sync.dma_start(out=wpe_t, in_=wpe_in[t])
            nc.vector.tensor_tensor(out=x[:, t, :], in0=wpe_t[:, :],
                                    in1=emb[:, :], op=ALU.add)

        def layernorm(src_ap, dst_ap, plo=0, n_rows=P):
            # src [n_rows, D] fp32 -> dst [n_rows, D] bf16 (gamma folded into W)
            phi = plo + n_rows
            ngrp = max(1, D // 512)
            st = stat.tile([P, ngrp, 6], F32, tag="bnst")
            grp = src_ap.rearrange("p (g d) -> p g d", g=ngrp)
            for g in range(ngrp):
                nc.vector.bn_stats(out=st[plo:phi, g, :], in_=grp[:, g, :])
            mv = stat.tile([P, 2], F32, tag="bnmv")
            nc.vector.bn_aggr(out=mv[plo:phi], in_=st[plo:phi])
            rs = stat.tile([P, 1], F32, tag="bnrs")
            nc.scalar.activation(out=rs[plo:phi], in_=mv[plo:phi, 1:2],
                                 func=AF.Sqrt, bias=eps_t[plo:phi])
            nc.vector.reciprocal(out=rs[plo:phi], in_=rs[plo:phi])
            nc.vector.tensor_scalar(out=dst_ap, in0=src_ap,
                                    scalar1=mv[plo:phi, 0:1], scalar2=rs[plo:phi],
                                    op0=ALU.subtract, op1=ALU.mult)

        def transpose_to(h_bf, dstT, psp):
            # h_bf [P, NBL, D] bf16 token-major -> dstT [P, NC, TL] feature-major
            for t in range(NBL):
                for cc in range(NC):
                    tp = psp.tile([P, P], BF16, tag="tp")
                    nc.tensor.transpose(out=tp[:, :], in_=h_bf[:, t, cc * P:(cc + 1) * P],
                                        identity=ident[:, :])
                    nc.scalar.copy(out=dstT[:, cc, t * P:(t + 1) * P], in_=tp[:, :])

        def load_w(dram_l):
            w = wpool.tile([P, NC, D], BF16, tag="w4")
            nc.sync.dma_start(out=w, in_=dram_l.rearrange("(c p) o -> p c o", p=P))
            return w

        # ================= layers =================
        for l in range(L):
            h_bf = sb3.tile([P, NBL, D], BF16, tag="h_bf")
            with tc.tile_pool(name=f"ps_tp{l}", bufs=2, space="PSUM") as psp:
                for t in range(NBL):
                    layernorm(x[:, t, :], h_bf[:, t, :])
                transpose_to(h_bf, hT, psp)

            # ---- K, V, Q projections ----
            with tc.tile_pool(name=f"ps_kvq{l}", bufs=3, space="PSUM") as psp, \
                 tc.tile_pool(name=f"ps_v{l}", bufs=1, space="PSUM") as psv:
                vk_view = k_in.ap().rearrange("(c p t) -> p c t", c=NC, p=P)
                vv_view = v_in.ap().rearrange("(t p d) -> p t d", t=NBL, p=P)
                wk = load_w(wk_all[l])
                for co in range(NC):
                    ps = psp.tile([P, TL], F32, tag="mm")
                    for cc in range(NC):
                        nc.tensor.matmul(ps[:, :], wk[:, cc, co * P:(co + 1) * P],
                                         hT[:, cc, :], start=(cc == 0),
                                         stop=(cc == NC - 1))
                    kst = sb3.tile([P, TL], BF16, tag="kvst")
                    nc.scalar.copy(out=kst[:, :], in_=ps[:, :])
                    nc.sync.dma_start(out=vk_view[:, co, :], in_=kst)
                nc.gpsimd.collective_compute(
                    "AllGather", ALU.bypass, replica_groups=pair_groups,
                    ins=[k_in.ap()], outs=[k_out.ap()])
                wv = load_w(wv_all[l])
                for t in range(NBL):
                    pss = [psv.tile([P, 512], F32, tag=f"mm2{i}",
                                    name=f"vps{l}_{t}_{i}")
                           for i in range(len(DHS))]
                    for cc in range(NC):
                        for i, (off, dw) in enumerate(DHS):
                            nc.tensor.matmul(pss[i][:, 0:dw],
                                             hT[:, cc, t * P:(t + 1) * P],
                                             wv[:, cc, off:off + dw],
                                             start=(cc == 0), stop=(cc == NC - 1),
                                             skip_group_check=True)
                    for i, (off, dw) in enumerate(DHS):
                        vst = sb3.tile([P, 512], BF16, tag="kvst")
                        nc.scalar.copy(out=vst[:, 0:dw], in_=pss[i][:, 0:dw])
                        nc.sync.dma_start(out=vv_view[:, t, off:off + dw],
                                          in_=vst[:, 0:dw])
                nc.gpsimd.collective_compute(
                    "AllGather", ALU.bypass, replica_groups=pair_groups,
                    ins=[v_in.ap()], outs=[v_out.ap()])
                wq = load_w(wq_all[l])
                for co in range(NC):
                    ps = psp.tile([P, TL], F32, tag="mm")
                    for cc in range(NC):
                        nc.tensor.matmul(ps[:, :], wq[:, cc, co * P:(co + 1) * P],
                                         hT[:, cc, :], start=(cc == 0),
                                         stop=(cc == NC - 1))
                    nc.scalar.copy(out=qT[:, co, :], in_=ps[:, :])

            # unpack gathered K/V into global order
            for g in range(NKT):
                r, slot = G2SLOT[g]
                ksrc = k_out[r].rearrange("(c p t) -> p c t", c=NC, p=P)
                nc.sync.dma_start(out=kT_full[:, :, g * P:(g + 1) * P],
                                  in_=ksrc[:, :, slot * P:(slot + 1) * P])
            for g in range(NKT):
                r, slot = G2SLOT[g]
                vsrc = v_out[r].rearrange("(t p h d) -> t p h d",
                                          t=NBL, p=P, h=H)
                nc.sync.dma_start(out=v_full[:, g, :, 0:HD], in_=vsrc[slot])

            # ---- attention (k-major scores; sums via augmented V) ----
            with tc.tile_pool(name=f"ps_at{l}", bufs=4, space="PSUM") as psp, \
                 tc.tile_pool(name=f"ps_ctx{l}", bufs=2, space="PSUM") as psc:
                for h in range(H):
                    cc = (h * HD) // P
                    po = (h * HD) % P
                    ctx_ps = psc.tile([HD + 1, TL], F32, tag="ctx")
                    for kt in range(NKT):
                        qoff = (kt // 2) * P
                        NQ = TL - qoff
                        s_ps = psp.tile([P, TL], F32, tag="s")
                        nc.tensor.matmul(
                            s_ps[:, 0:NQ],
                            kT_full[po:po + HD, cc, kt * P:(kt + 1) * P],
                            qT[po:po + HD, cc, qoff:TL], start=True, stop=True)
                        nc.vector.tensor_tensor(out=s_ps[:, 0:P], in0=s_ps[:, 0:P],
                                                in1=maskT[:, kt, :], op=ALU.add)
                        pr = probs_pool.tile([P, TL], BF16, tag="probs")
                        nc.scalar.activation(out=pr[:, 0:NQ], in_=s_ps[:, 0:NQ],
                                             func=AF.Exp,
                                             scale=1.0 / math.sqrt(HD))
                        nc.tensor.matmul(ctx_ps[:, qoff:TL],
                                         v_full[:, kt, h, :], pr[:, 0:NQ],
                                         start=(kt == 0), stop=(kt == NKT - 1),
                                         skip_group_check=True)
                    # normalize columns by the sums row (row HD of ctx_ps)
                    rsum = stat.tile([P, TL], F32, tag="rsum")
                    nc.vector.reciprocal(out=rsum[HD:HD + 1, :],
                                         in_=ctx_ps[HD:HD + 1, :])
                    rb_ps = psc.tile([HD, TL], F32, tag="rb")
                    nc.tensor.matmul(rb_ps[:, :], ones_col[HD:HD + 1, :],
                                     rsum[HD:HD + 1, :], start=True, stop=True)
                    rb = sb3.tile([HD, TL], F32, tag="rbs")
                    nc.scalar.copy(out=rb[:, :], in_=rb_ps[:, :])
                    nc.vector.tensor_tensor(out=ctxT[po:po + HD, cc, :],
                                            in0=ctx_ps[0:HD, :], in1=rb[:, :],
                                            op=ALU.mult)

            # ---- output projection + residual ----
            with tc.tile_pool(name=f"ps_o{l}", bufs=2, space="PSUM") as psp:
                wo = load_w(wo_all[l])
                for t in range(NBL):
                    pss = [psp.tile([P, 512], F32, tag=f"mm2{i}",
                                    name=f"ops{l}_{t}_{i}")
                           for i in range(len(DHS))]
                    for cc2 in range(NC):
                        for i, (off, dw) in enumerate(DHS):
                            nc.tensor.matmul(pss[i][:, 0:dw],
                                             ctxT[:, cc2, t * P:(t + 1) * P],
                                             wo[:, cc2, off:off + dw],
                                             start=(cc2 == 0), stop=(cc2 == NC - 1),
                                             skip_group_check=True)
                    for i, (off, dw) in enumerate(DHS):
                        nc.vector.tensor_tensor(
                            out=x[:, t, off:off + dw],
                            in0=x[:, t, off:off + dw], in1=pss[i][:, 0:dw],
                            op=ALU.add)

            # ---- MLP ----
            h2 = sb3.tile([P, NBL, D], BF16, tag="h_bf")
            with tc.tile_pool(name=f"ps_tp2{l}", bufs=2, space="PSUM") as psp:
                for t in range(NBL):
                    layernorm(x[:, t, :], h2[:, t, :])
                transpose_to(h2, hT, psp)

            wfc_src = wfc_all[l].rearrange("(c p) o -> p c o", p=P)
            wpr_src = wpr_all[l].rearrange("(g p) o -> p g o", p=P)
            for rnd in range(NROUND):
                with tc.tile_pool(name=f"ps_mlp{l}_{rnd}", bufs=3, space="PSUM") as psp:
                    uT = upool.tile([P, RM, TL], BF16, tag="uT",
                                    name=f"uT_{l}_{rnd}")
                    for mgl in range(RM // 4):
                        mg = rnd * (RM // 4) + mgl
                        wfc = wpool.tile([P, NC, 512], BF16, tag="wstream")
                        nc.sync.dma_start(out=wfc,
                                          in_=wfc_src[:, :, mg * 512:(mg + 1) * 512])
                        for m in range(4):
                            ps = psp.tile([P, TL], F32, tag="mm")
                            for cc in range(NC):
                                nc.tensor.matmul(ps[:, :], wfc[:, cc, m * P:(m + 1) * P],
                                                 hT[:, cc, :], start=(cc == 0),
                                                 stop=(cc == NC - 1))
                            if cfg.get("gelu") == "sigmoid":
                                sg = sb3.tile([P, TL], F32, tag="sg")
                                nc.scalar.activation(out=sg[:, :], in_=ps[:, :],
                                                     func=AF.Sigmoid, scale=1.702)
                                nc.vector.tensor_tensor(out=uT[:, mgl * 4 + m, :],
                                                        in0=ps[:, :], in1=sg[:, :],
                                                        op=ALU.mult)
                            else:
                                nc.scalar.activation(out=uT[:, mgl * 4 + m, :],
                                                     in_=ps[:, :], func=AF.Gelu)
                    for off, dw in DHS:
                        wpr = wpool.tile([P, RM, 512], BF16, tag="wstream")
                        nc.sync.dma_start(
                            out=wpr[:, :, 0:dw],
                            in_=wpr_src[:, rnd * RM:(rnd + 1) * RM, off:off + dw])
                        for t in range(NBL):
                            ps = psp.tile([P, 512], F32, tag="mm2")
                            for hh in range(RM):
                                nc.tensor.matmul(
                                    ps[:, 0:dw], uT[:, hh, t * P:(t + 1) * P],
                                    wpr[:, hh, 0:dw], start=(hh == 0),
                                    stop=(hh == RM - 1))
                            nc.vector.tensor_tensor(
                                out=x[:, t, off:off + dw],
                                in0=x[:, t, off:off + dw], in1=ps[:, 0:dw],
                                op=ALU.add)

        # ================= final LN + lm head =================
        xrow = lm1.tile([1, D], F32, tag="xrow")
        nc.gpsimd.dma_start(out=xrow[0:1, :], in_=x[P - 1:P, NBL - 1, :])
        xln = lm1.tile([P, D], F32, tag="xln")
        layernorm(xrow[0:1, :], xln[0:1, :], plo=0, n_rows=1)
        xl0 = lm1.tile([1, D], BF16, tag="xl0")
        nc.vector.tensor_copy(out=xl0[0:1, :], in_=xln[0:1, :])
        oh = stat.tile([1, B], BF16, tag="oh")
        nc.sync.dma_start(out=oh, in_=onehot_in.ap())
        contrib = lm1.tile([B, D], F32, tag="contrib")
        with tc.tile_pool(name="ps_lm", bufs=4, space="PSUM") as psp:
            for off, dw in DHS:
                ps = psp.tile([B, 512], F32, tag="lmps")
                nc.tensor.matmul(ps[:, 0:dw], oh[0:1, :],
                                 xl0[0:1, off:off + dw],
                                 start=True, stop=True)
                nc.scalar.copy(out=contrib[:, off:off + dw], in_=ps[:, 0:dw])
            nc.sync.dma_start(out=lm_in.ap(), in_=contrib)
            nc.gpsimd.collective_compute(
                "AllReduce", ALU.add, replica_groups=all_group,
                ins=[lm_in.ap()], outs=[lm_out.ap()])
            xl_all = lm1.tile([B, D], F32, tag="xl_all")
            nc.sync.dma_start(out=xl_all, in_=lm_out.ap())
            xl_bf = lm1.tile([B, D], BF16, tag="xl_bf")
            nc.vector.tensor_copy(out=xl_bf[:, :], in_=xl_all[:, :])
            xlT = lm1.tile([P, NC, B], BF16, tag="xlT")
            for cc in range(NC):
                tp = psp.tile([P, B], BF16, tag="lmtp")
                nc.tensor.transpose(out=tp[:, :], in_=xl_bf[:, cc * P:(cc + 1) * P],
                                    identity=ident[0:B, 0:B])
                nc.scalar.copy(out=xlT[:, cc, :], in_=tp[:, :])
            for vt in range(NVT):
                nv = min(512, VS - vt * 512)
                wvt = wpool.tile([P, NC, 512], BF16, tag="wstream")
                src = wteT_in.rearrange("(c p) v -> p c v", p=P)
                nc.sync.dma_start(out=wvt[:, :, 0:nv],
                                  in_=src[:, :, vt * 512:vt * 512 + nv])
                ps = psp.tile([B, 512], F32, tag="lmps")
                for cc in range(NC):
                    nc.tensor.matmul(ps[:, 0:nv], xlT[:, cc, :], wvt[:, cc, 0:nv],
                                     start=(cc == 0), stop=(cc == NC - 1))
                lmout = sb3.tile([B, 512], F32, tag="lmout")
                nc.scalar.copy(out=lmout[:, 0:nv], in_=ps[:, 0:nv])
                nc.sync.dma_start(out=logits_out.ap()[:, vt * 512:vt * 512 + nv],
                                  in_=lmout[:, 0:nv])

    nc.compile()
    return nc


# --------------------------------------------------------------------------
# host side
# --------------------------------------------------------------------------

def _bf16(a):
    return np.asarray(a, dtype=np.float32).astype(ml_dtypes.bfloat16)


def shard_inputs(cfg, inputs):
    c = derived(cfg)
    B, T, D, L, V, NCORES = cfg["B"], cfg["T"], cfg["D"], cfg["L"], cfg["V"], cfg["NCORES"]
    VS = c["VS"]
    idx = np.asarray(inputs["idx"]).astype(np.int32)
    wte = np.asarray(inputs["wte"], dtype=np.float32)
    wpe = np.asarray(inputs["wpe"], dtype=np.float32)
    ln1_g = np.asarray(inputs["ln1_g"], dtype=np.float32)
    ln2_g = np.asarray(inputs["ln2_g"], dtype=np.float32)
    lnf_g = np.asarray(inputs["lnf_g"], dtype=np.float32)

    # fold LN gammas into the consuming weights (exact rewrite)
    wq = _bf16(inputs["wq"] * ln1_g[:, :, None])
    wk = _bf16(inputs["wk"] * ln1_g[:, :, None])
    wv = _bf16(inputs["wv"] * ln1_g[:, :, None])
    wo = _bf16(inputs["wo"])
    wfc = _bf16(inputs["wfc"] * ln2_g[:, :, None])
    wpr = _bf16(inputs["wpr"])
    wte_emb = _bf16(wte)
    wteT = wte.T * lnf_g[:, None]            # [D, V]
    wteT_pad = np.zeros((D, VS * NCORES), dtype=np.float32)
    wteT_pad[:, :V] = wteT
    wteT_pad = _bf16(wteT_pad)

    in_maps = []
    for core in range(NCORES):
        pair, rank = divmod(core, 2)
        G = G_EVEN if rank == 0 else G_ODD
        rows = np.concatenate([np.arange(g * P, (g + 1) * P) for g in G])
        # k-major boundary masks: for k-tile kt the first valid q block is kt//2
        maskT = np.zeros((c["NKT"], P, P), dtype=np.float32)
        for kt in range(c["NKT"]):
            jb = kt // 2
            k_abs = kt * P + np.arange(P)[:, None]
            q_abs = G[jb] * P + np.arange(P)[None, :]
            maskT[kt] = np.where(k_abs <= q_abs, 0.0, NEG)
        onehot = np.zeros((1, B), dtype=np.float32)
        if rank == 0:                         # owns global block 7 (last token)
            onehot[0, pair] = 1.0
        in_maps.append({
            "x_idx": idx[pair][rows].reshape(c["NBL"], P),
            "wpe_loc": wpe[rows].reshape(c["NBL"], P, D),
            "wte_emb": wte_emb,
            "wq_all": wq, "wk_all": wk, "wv_all": wv, "wo_all": wo,
            "wfc_all": wfc, "wpr_all": wpr,
            "maskT": maskT,
            "onehot": _bf16(onehot),
            "wteT_loc": np.ascontiguousarray(wteT_pad[:, core * VS:(core + 1) * VS]),
        })
    return in_maps


def assemble(cfg, results):
    c = derived(cfg)
    V, VS = cfg["V"], c["VS"]
    logits = np.concatenate([r["logits_loc"] for r in results], axis=1)
    return np.ascontiguousarray(logits[:, :V]).astype(np.float32)


def kernel(**inputs):
    from concourse import bass_utils
    cfg = full_cfg()
    nc = build_program(cfg)
    in_maps = shard_inputs(cfg, inputs)
    res = bass_utils.run_bass_kernel_spmd(nc, in_maps,
                                          core_ids=list(range(cfg["NCORES"])))
    return assemble(cfg, res.results)
